# revision 49
# baseline (speedup 1.0000x reference)
"""Trainium2 Bass kernel for causal multi-head attention with RoPE.

Sharding: tensor-parallel over heads. 16 heads / 8 cores = 2 heads per core.
Each core computes QKV projection for its 2 heads (full sequence), RoPE,
causal flash-style attention, and the output-projection row slices belonging
to its heads (the reference's permute/reshape makes output rows
head-partitioned, so no cross-core reduction is needed).

All matmul operands are bf16 (fp32 PSUM accumulation): halves HBM traffic
and LDWEIGHTS time vs fp32, and runs 1 cycle/row at any moving-dim width so
causal diagonal blocks can use partial-width matmuls. Scores are computed
transposed (keys on partitions) so softmax normalization is a ones-matmul
partition-sum and attn@V needs no transposes. Projection matmuls stream the
full 512-col PSUM bank width; per-512-token rot/v tiles keep attention
dependencies block-granular; the softmax denominator is accumulated in
groups of four on DVE with a single delayed ones-matmul per group.
"""

import math
import os
import sys

for _p in ("/opt/trn_rl_repo",):
    if _p not in sys.path and os.path.isdir(_p):
        sys.path.insert(0, _p)

import numpy as np
import ml_dtypes

import concourse.bass as bass  # noqa: F401  (AP helpers)
import concourse.mybir as mybir
import concourse.tile as tile
from concourse import bacc
from concourse.bass_utils import run_bass_kernel_spmd

F32 = mybir.dt.float32
BF16 = mybir.dt.bfloat16
NPBF = np.dtype(ml_dtypes.bfloat16)

B, T, C = 2, 2048, 2048
H, D = 16, 128
N_CORES = 8
HPC = H // N_CORES          # heads per core (2)
BT = B * T                  # 4096
KC = C // 128               # 16 contraction blocks
TB = 512                    # token block (attention q-block and projection)
NTB = T // TB               # 4 t-blocks per batch
SCALE = 1.0 / math.sqrt(D)
LAG = 2                     # score->exp->attn pipeline depth (in 128-kt units)

_CACHED_NC = None


def build_nc():
    nc = bacc.Bacc("TRN2", target_bir_lowering=False)

    xT = nc.dram_tensor("xT", [C, BT], BF16, kind="ExternalInput")
    wqkT = nc.dram_tensor("wqkT", [C, 4 * 128], BF16, kind="ExternalInput")
    wvT = nc.dram_tensor("wvT", [C, 2 * 128], BF16, kind="ExternalInput")
    owF = nc.dram_tensor("owF", [C, C], BF16, kind="ExternalInput")
    cosF = nc.dram_tensor("cosF", [128, T], BF16, kind="ExternalInput")
    sinS = nc.dram_tensor("sinS", [128, T], BF16, kind="ExternalInput")
    onesI = nc.dram_tensor("onesI", [128, 128], BF16, kind="ExternalInput")
    maskI = nc.dram_tensor("maskI", [128, 128], BF16, kind="ExternalInput")
    y = nc.dram_tensor("y", [B * HPC, 128, C], F32, kind="ExternalOutput")

    with tile.TileContext(nc) as tc:
        with tc.tile_pool(name="wpool", bufs=1) as wpool, \
             tc.tile_pool(name="xpool", bufs=16) as xpool, \
             tc.tile_pool(name="dpool", bufs=2) as dpool, \
             tc.tile_pool(name="rotpool", bufs=1) as rotpool, \
             tc.tile_pool(name="vpool", bufs=1) as vpool, \
             tc.tile_pool(name="apool", bufs=1) as apool, \
             tc.tile_pool(name="epool", bufs=8) as epool, \
             tc.tile_pool(name="tpool", bufs=2) as tpool, \
             tc.tile_pool(name="rpool", bufs=2) as rpool, \
             tc.tile_pool(name="ypool", bufs=2) as ypool, \
             tc.tile_pool(name="flowps", bufs=5, space="PSUM") as flowps, \
             tc.tile_pool(name="attps", bufs=2, space="PSUM") as attps, \
             tc.tile_pool(name="denps", bufs=1, space="PSUM") as denps:

            twqk = wpool.tile([128, KC, 4 * 128], BF16)
            twv = wpool.tile([128, KC, 2 * 128], BF16)
            tcf = wpool.tile([128, T], BF16)
            tsn = wpool.tile([128, T], BF16)
            tones = wpool.tile([128, 128], BF16)
            tmask = wpool.tile([128, 128], BF16)
            tow = wpool.tile([128, KC, C], BF16)
            wqkr = wqkT.rearrange("(kb p) m -> p kb m", p=128)
            wvr = wvT.rearrange("(kb p) m -> p kb m", p=128)
            owr = owF.rearrange("(u p) j -> p u j", p=128)
            # Startup: the scalar (ACT) DMA queue boots ~3.4us before the
            # sync queue, so the first token-block's even x chunks ride it,
            # interleaved with the twqk chunks the first matmul chain
            # needs; odd chunks go on sync. Both queues then feed the first
            # chain in parallel. out_w is dispatched later in small per-u
            # slices interleaved with the b0 RoPE copies so no single long
            # dispatch blocks the ACT queue.
            xg00 = [xpool.tile([128, 2, TB], BF16, tag="xk", name="xg")
                    for _ in range(KC // 2)]
            xg01 = [xpool.tile([128, 2, TB], BF16, tag="xk", name="xg")
                    for _ in range(KC // 2)]
            xTr0 = xT[:, 0:TB].rearrange("(kb p) t -> p kb t", p=128)
            xTr1 = xT[:, TB:2 * TB].rearrange("(kb p) t -> p kb t", p=128)
            nc.scalar.dma_start(xg00[0][:], xTr0[:, 0:2, :])
            nc.scalar.dma_start(twqk[:, 0:4, :], wqkr[:, 0:4, :])
            nc.scalar.dma_start(xg00[2][:], xTr0[:, 4:6, :])
            nc.scalar.dma_start(twqk[:, 4:8, :], wqkr[:, 4:8, :])
            nc.scalar.dma_start(xg00[4][:], xTr0[:, 8:10, :])
            nc.scalar.dma_start(xg00[6][:], xTr0[:, 12:14, :])
            for g in (1, 3, 5, 7):
                nc.sync.dma_start(xg00[g][:],
                                  xTr0[:, 2 * g:2 * (g + 1), :])
            for kh in range(2, 4):
                nc.scalar.dma_start(twqk[:, kh * 4:(kh + 1) * 4, :],
                                    wqkr[:, kh * 4:(kh + 1) * 4, :])
            nc.scalar.dma_start(tcf[:], cosF[:, :])
            nc.scalar.dma_start(tsn[:], sinS[:, :])
            for g in (0, 2, 4, 6):
                nc.scalar.dma_start(xg01[g][:],
                                    xTr1[:, 2 * g:2 * (g + 1), :])
            for g in (1, 3, 5, 7):
                nc.sync.dma_start(xg01[g][:],
                                  xTr1[:, 2 * g:2 * (g + 1), :])
            nc.scalar.dma_start(twv[:], wvr[:, :, :])
            nc.scalar.dma_start(tones[:], onesI[:, :])
            nc.scalar.dma_start(tmask[:], maskI[:, :])

            # -------- output projection (per-head 128-row slices) --------
            # reference applies permute(0,2,1,3).reshape(B,T,C) to a
            # [B,T,H,D] tensor: out row t' = h*128 + t//16 uses head h,
            # col c' = (t%16)*128 + d.  Y_slice = attn_h.reshape(128,
            # 16*128) @ out_w.T, contracting over (u=t%16, d).  One chunk
            # is one jb (512 out cols); chunks are emitted interleaved with
            # the NEXT head's attention so the PE-only projection hides the
            # exp-bound tail of the attention pipeline.
            def emit_op(at, bb, h, jb, last=False):
                av = at[:].rearrange("p (a u) -> p a u", u=16)
                psy = flowps.tile([128, 512], F32, tag="flow")
                for u in range(KC):
                    nc.tensor.matmul(
                        psy[:], av[:, :, u],
                        tow[:, u, jb * 512:(jb + 1) * 512],
                        start=(u == 0), stop=(u == KC - 1))
                # ys copy on DVE (projection must not touch the ACT queue,
                # which is saturated by exp during the paired attention);
                # y DMA rides the scalar queue so it never delays x
                # prefetch on the sync queue.
                for piece in range(2 if last else 1):
                    w0p = piece * 256 if last else 0
                    w1p = 256 + piece * 256 if last else 512
                    ys = ypool.tile([128, 512], F32, tag="ys")
                    nc.vector.tensor_copy(ys[:, w0p:w1p], psy[:, w0p:w1p])
                    nc.scalar.dma_start(
                        y[bb * HPC + h, :, jb * 512 + w0p:jb * 512 + w1p],
                        ys[:, w0p:w1p])

            for b in range(B):
                # ---------------- QKV projection + RoPE ----------------
                # rots/vts are split per 512-token block so attention's
                # dependencies are block-granular (a monolithic tile would
                # make the first score matmul wait for the last RoPE chain).
                rots = [[rotpool.tile([128, TB], BF16, tag=f"rot{m}_{j}",
                                      name=f"rot{m}_{j}")
                         for j in range(NTB)] for m in range(4)]
                vts = [vpool.tile([128, 4, 2 * 128], BF16, tag=f"vt{j}",
                                  name=f"vt{j}") for j in range(NTB)]
                for tb in range(NTB):
                    c0 = b * T + tb * TB
                    ts_sl = slice(tb * TB, (tb + 1) * TB)
                    xTr = xT[:, c0:c0 + TB].rearrange(
                        "(kb p) t -> p kb t", p=128)
                    if b == 0 and tb == 0:
                        xgs = xg00  # preloaded across both DMA queues
                    elif b == 0 and tb == 1:
                        xgs = xg01
                    else:
                        xgs = []
                        for g in range(KC // 2):
                            xg = xpool.tile([128, 2, TB], BF16, tag="xk",
                                            name="xg")
                            nc.sync.dma_start(xg[:],
                                              xTr[:, g * 2:(g + 1) * 2, :])
                            xgs.append(xg)
                    xk = [xgs[k // 2][:, k % 2, :] for k in range(KC)]
                    for m in range(4):
                        ps = flowps.tile([128, TB], F32, tag="flow")
                        for k in range(KC):
                            nc.tensor.matmul(
                                ps[:], twqk[:, k, m * 128:(m + 1) * 128],
                                xk[k], start=(k == 0), stop=(k == KC - 1))
                        # RoPE: rows 0:64 = x1, 64:128 = x2 of this head
                        qsb = tpool.tile([128, TB], BF16, tag="qsb")
                        nc.scalar.copy(qsb[:], ps[:])
                        if b == 0:
                            u = tb * 4 + m
                            nc.scalar.dma_start(tow[:, u, :], owr[:, u, :])
                        qsw = tpool.tile([128, TB], BF16, tag="qsw")
                        nc.gpsimd.dma_start(qsw[0:64, :], qsb[64:128, :])
                        nc.gpsimd.dma_start(qsw[64:128, :], qsb[0:64, :])
                        pc = tpool.tile([128, TB], BF16, tag="pc")
                        nc.vector.tensor_mul(out=pc[:], in0=qsb[:],
                                             in1=tcf[:, ts_sl])
                        pn = tpool.tile([128, TB], BF16, tag="pn")
                        nc.vector.tensor_mul(out=pn[:], in0=qsw[:],
                                             in1=tsn[:, ts_sl])
                        nc.vector.tensor_add(
                            out=rots[m][tb][:], in0=pc[:], in1=pn[:])
                    for vp in range(2):
                        psv = flowps.tile([128, TB], F32, tag="flow")
                        for half in range(2):
                            ts = vp * 2 + half
                            for k in range(KC):
                                nc.tensor.matmul(
                                    psv[:, half * 256:(half + 1) * 256],
                                    xk[k][:, ts * 128:(ts + 1) * 128],
                                    twv[:, k, :], start=(k == 0),
                                    stop=(k == KC - 1))
                        nc.vector.tensor_copy(
                            vts[tb][:, vp * 2:(vp + 1) * 2, :],
                            psv[:].rearrange("p (v c) -> p v c", v=2))

                # ---------------- attention ----------------
                atn = [apool.tile([128, T], BF16, tag=f"attnT{h}",
                                  name=f"attnT{h}") for h in range(HPC)]

                def emit_att(h, tb):
                    if True:
                        ts_sl = slice(tb * TB, (tb + 1) * TB)
                        ns = (tb + 1) * (TB // 128)
                        ng = ns // 4
                        ps_att = attps.tile([128, TB], F32, tag="psatt")
                        ps_den = denps.tile([128, TB], F32, tag="psden")
                        dacc = [None] * ng
                        den_ready = []

                        def den_mm(g):
                            nc.tensor.matmul(
                                ps_den[:], tones[:], dacc[g][:],
                                start=(g == 0), stop=(g == ng - 1))

                        def flush(ep, p, w):
                            # softmax denominator: group 4 et blocks with
                            # (partial-width) DVE adds, one ones-matmul per
                            # group, emitted one group late so the PE never
                            # waits on the DVE accumulation chain. Every
                            # group-start block is full-width (r <= 0).
                            g = p // 4
                            if p % 4 == 0:
                                dacc[g] = dpool.tile([128, TB], BF16,
                                                     tag="dacc", name="dacc")
                                nc.vector.tensor_copy(dacc[g][:], ep[:])
                            else:
                                nc.vector.tensor_add(
                                    out=dacc[g][:, w:], in0=dacc[g][:, w:],
                                    in1=ep[:, w:])
                            if p % 4 == 3:
                                den_ready.append(g)
                                if len(den_ready) > 1:
                                    den_mm(den_ready.pop(0))
                            nc.tensor.matmul(
                                ps_att[:, w:],
                                vts[p // 4][:, p % 4,
                                            h * 128:(h + 1) * 128],
                                ep[:, w:], start=(p == 0),
                                stop=(p == ns - 1))

                        pend = []
                        for si in range(ns):
                            # cols < r are fully causal-masked for this
                            # kt-block; compute only [w0:]
                            r = si * 128 - tb * TB
                            w0 = max(r, 0)
                            ps_sc = flowps.tile([128, TB], F32, tag="flow")
                            nc.tensor.matmul(
                                ps_sc[:, w0:],
                                rots[2 + h][si // 4][
                                    :, (si % 4) * 128:(si % 4 + 1) * 128],
                                rots[h][tb][:, w0:],
                                start=True, stop=True)
                            et = epool.tile([128, TB], BF16, tag="et")
                            nc.scalar.activation(
                                et[:, w0:], ps_sc[:, w0:],
                                mybir.ActivationFunctionType.Exp,
                                scale=SCALE)
                            if r >= 0:
                                # diagonal block: zero the sub-diagonal
                                # triangle of [w0:w0+128] by multiplying
                                # with a precomputed upper-triangle mask on
                                # DVE (keeps gpsimd off the attention
                                # critical path); cols [0:w0] stay garbage
                                # but every consumer reads [w0:] (the
                                # group's p%4==0 full-width dacc copy is
                                # always the r==0 block)
                                nc.vector.tensor_mul(
                                    out=et[:, w0:w0 + 128],
                                    in0=et[:, w0:w0 + 128],
                                    in1=tmask[:])
                            pend.append((et, si, w0))
                            if len(pend) > LAG:
                                flush(*pend.pop(0))
                        for args in pend:
                            flush(*args)
                        for g in den_ready:
                            den_mm(g)
                        rcp = rpool.tile([128, TB], F32, tag="rcp")
                        nc.vector.reciprocal_approx_fast(out=rcp[:],
                                                         in_=ps_den[:])
                        nc.vector.tensor_mul(
                            out=atn[h][:, ts_sl], in0=ps_att[:], in1=rcp[:])

                for h in range(HPC):
                    for tb in range(NTB):
                        emit_att(h, tb)
                for h in range(HPC):
                    for jb in range(4):
                        emit_op(atn[h], b, h, jb,
                                last=(b == B - 1 and h == HPC - 1
                                      and jb == 3))
    nc.compile()
    return nc


def _get_nc():
    global _CACHED_NC
    if _CACHED_NC is None:
        _CACHED_NC = build_nc()
    return _CACHED_NC


def _rope_tables():
    pos = np.arange(T, dtype=np.float64)[:, None]
    div = np.exp(np.arange(0, D, 2, dtype=np.float64) *
                 (-math.log(10000.0) / D))
    ang = pos * div  # [T, 64]
    sinT = np.sin(ang).T.astype(np.float32)  # [64, T]
    cosT = np.cos(ang).T.astype(np.float32)
    cosF = np.ascontiguousarray(np.concatenate([cosT, cosT], axis=0))
    sinS = np.ascontiguousarray(np.concatenate([-sinT, sinT], axis=0))
    return cosF.astype(NPBF), sinS.astype(NPBF)


def make_in_maps(x, qkv_w, out_w):
    xT = np.ascontiguousarray(x.reshape(BT, C).T).astype(NPBF)
    owF = np.ascontiguousarray(out_w.T).astype(NPBF)
    cosF, sinS = _rope_tables()
    ones = np.ones((128, 128), dtype=NPBF)
    mask = np.triu(np.ones((128, 128), dtype=np.float32)).astype(NPBF)
    in_maps = []
    for c in range(N_CORES):
        h0, h1 = 2 * c, 2 * c + 1
        wqk = np.concatenate([
            qkv_w[h0 * D:(h0 + 1) * D],
            qkv_w[h1 * D:(h1 + 1) * D],
            qkv_w[C + h0 * D:C + (h0 + 1) * D],
            qkv_w[C + h1 * D:C + (h1 + 1) * D],
        ], axis=0)                       # [512, 2048]
        wv = np.concatenate([
            qkv_w[2 * C + h0 * D:2 * C + (h0 + 1) * D],
            qkv_w[2 * C + h1 * D:2 * C + (h1 + 1) * D],
        ], axis=0)                       # [256, 2048]
        in_maps.append({
            "xT": xT,
            "wqkT": np.ascontiguousarray(wqk.T).astype(NPBF),
            "wvT": np.ascontiguousarray(wv.T).astype(NPBF),
            "owF": owF,
            "cosF": cosF,
            "sinS": sinS,
            "onesI": ones,
            "maskI": mask,
        })
    return in_maps


def kernel(x, qkv_w, out_w, _trace=False, _trace_kwargs=None):
    x = np.asarray(x, dtype=np.float32)
    qkv_w = np.asarray(qkv_w, dtype=np.float32)
    out_w = np.asarray(out_w, dtype=np.float32)
    nc = _get_nc()
    in_maps = make_in_maps(x, qkv_w, out_w)
    kwargs = {}
    if _trace:
        kwargs["trace"] = True
        if _trace_kwargs:
            kwargs.update(_trace_kwargs)
    res = run_bass_kernel_spmd(nc, in_maps, core_ids=list(range(N_CORES)),
                               **kwargs)
    out = np.empty((B, T, C), dtype=np.float32)
    for c in range(N_CORES):
        yc = res.results[c]["y"]  # [B*HPC, 128, C]
        for b in range(B):
            for hl in range(HPC):
                hg = HPC * c + hl
                out[b, hg * 128:(hg + 1) * 128] = yc[b * HPC + hl]
    if _trace:
        return out, res
    return out


# revision 53
# speedup vs baseline: 1.0126x; 1.0126x over previous
"""Trainium2 Bass kernel for causal multi-head attention with RoPE.

Sharding: tensor-parallel over heads. 16 heads / 8 cores = 2 heads per core.
Each core computes QKV projection for its 2 heads (full sequence), RoPE,
causal flash-style attention, and the output-projection row slices belonging
to its heads (the reference's permute/reshape makes output rows
head-partitioned, so no cross-core reduction is needed).

All matmul operands are bf16 (fp32 PSUM accumulation): halves HBM traffic
and LDWEIGHTS time vs fp32, and runs 1 cycle/row at any moving-dim width so
causal diagonal blocks can use partial-width matmuls. Scores are computed
transposed (keys on partitions) so softmax normalization is a ones-matmul
partition-sum and attn@V needs no transposes. Projection matmuls stream the
full 512-col PSUM bank width; per-512-token rot/v tiles keep attention
dependencies block-granular; the softmax denominator is accumulated in
groups of four on DVE with a single delayed ones-matmul per group.
"""

import math
import os
import sys

for _p in ("/opt/trn_rl_repo",):
    if _p not in sys.path and os.path.isdir(_p):
        sys.path.insert(0, _p)

import numpy as np
import ml_dtypes

import concourse.bass as bass  # noqa: F401  (AP helpers)
import concourse.mybir as mybir
import concourse.tile as tile
from concourse import bacc
from concourse.bass_utils import run_bass_kernel_spmd

F32 = mybir.dt.float32
BF16 = mybir.dt.bfloat16
NPBF = np.dtype(ml_dtypes.bfloat16)

B, T, C = 2, 2048, 2048
H, D = 16, 128
N_CORES = 8
HPC = H // N_CORES          # heads per core (2)
BT = B * T                  # 4096
KC = C // 128               # 16 contraction blocks
TB = 512                    # token block (attention q-block and projection)
NTB = T // TB               # 4 t-blocks per batch
SCALE = 1.0 / math.sqrt(D)
LAG = 2                     # score->exp->attn pipeline depth (in 128-kt units)

_CACHED_NC = None


def build_nc():
    nc = bacc.Bacc("TRN2", target_bir_lowering=False)

    xT = nc.dram_tensor("xT", [C, BT], BF16, kind="ExternalInput")
    wqkT = nc.dram_tensor("wqkT", [C, 4 * 128], BF16, kind="ExternalInput")
    wvT = nc.dram_tensor("wvT", [C, 2 * 128], BF16, kind="ExternalInput")
    owF = nc.dram_tensor("owF", [C, C], BF16, kind="ExternalInput")
    cosF = nc.dram_tensor("cosF", [128, T], BF16, kind="ExternalInput")
    sinS = nc.dram_tensor("sinS", [128, T], BF16, kind="ExternalInput")
    onesI = nc.dram_tensor("onesI", [128, 128], BF16, kind="ExternalInput")
    maskI = nc.dram_tensor("maskI", [128, 128], BF16, kind="ExternalInput")
    y = nc.dram_tensor("y", [B * HPC, 128, C], F32, kind="ExternalOutput")

    with tile.TileContext(nc) as tc:
        with tc.tile_pool(name="wpool", bufs=1) as wpool, \
             tc.tile_pool(name="xpool", bufs=16) as xpool, \
             tc.tile_pool(name="dpool", bufs=2) as dpool, \
             tc.tile_pool(name="rotpool", bufs=1) as rotpool, \
             tc.tile_pool(name="vpool", bufs=1) as vpool, \
             tc.tile_pool(name="apool", bufs=1) as apool, \
             tc.tile_pool(name="epool", bufs=8) as epool, \
             tc.tile_pool(name="tpool", bufs=2) as tpool, \
             tc.tile_pool(name="rpool", bufs=2) as rpool, \
             tc.tile_pool(name="ypool", bufs=2) as ypool, \
             tc.tile_pool(name="flowps", bufs=5, space="PSUM") as flowps, \
             tc.tile_pool(name="attps", bufs=2, space="PSUM") as attps, \
             tc.tile_pool(name="denps", bufs=1, space="PSUM") as denps:

            twqk = wpool.tile([128, KC, 4 * 128], BF16)
            twv = wpool.tile([128, KC, 2 * 128], BF16)
            tcf = wpool.tile([128, T], BF16)
            tsn = wpool.tile([128, T], BF16)
            tones = wpool.tile([128, 128], BF16)
            tmask = wpool.tile([128, 128], BF16)
            tow = wpool.tile([128, KC, C], BF16)
            wqkr = wqkT.rearrange("(kb p) m -> p kb m", p=128)
            wvr = wvT.rearrange("(kb p) m -> p kb m", p=128)
            owr = owF.rearrange("(u p) j -> p u j", p=128)
            # Startup: the scalar (ACT) DMA queue boots ~3.4us before the
            # sync queue, so the first token-block's even x chunks ride it,
            # interleaved with the twqk chunks the first matmul chain
            # needs; odd chunks go on sync. Both queues then feed the first
            # chain in parallel. out_w is dispatched later in small per-u
            # slices interleaved with the b0 RoPE copies so no single long
            # dispatch blocks the ACT queue.
            xg00 = [xpool.tile([128, 2, TB], BF16, tag="xk", name="xg")
                    for _ in range(KC // 2)]
            xTr0 = xT[:, 0:TB].rearrange("(kb p) t -> p kb t", p=128)
            nc.scalar.dma_start(xg00[0][:], xTr0[:, 0:2, :])
            nc.scalar.dma_start(twqk[:, 0:4, :], wqkr[:, 0:4, :])
            nc.scalar.dma_start(xg00[2][:], xTr0[:, 4:6, :])
            nc.scalar.dma_start(twqk[:, 4:8, :], wqkr[:, 4:8, :])
            nc.scalar.dma_start(xg00[4][:], xTr0[:, 8:10, :])
            nc.scalar.dma_start(xg00[6][:], xTr0[:, 12:14, :])
            for g in (1, 3, 5, 7):
                nc.sync.dma_start(xg00[g][:],
                                  xTr0[:, 2 * g:2 * (g + 1), :])
            for kh in range(2, 4):
                nc.scalar.dma_start(twqk[:, kh * 4:(kh + 1) * 4, :],
                                    wqkr[:, kh * 4:(kh + 1) * 4, :])
            nc.scalar.dma_start(tcf[:], cosF[:, :])
            nc.scalar.dma_start(tsn[:], sinS[:, :])
            nc.scalar.dma_start(twv[:], wvr[:, :, :])
            nc.scalar.dma_start(tones[:], onesI[:, :])
            nc.scalar.dma_start(tmask[:], maskI[:, :])

            # -------- output projection (per-head 128-row slices) --------
            # reference applies permute(0,2,1,3).reshape(B,T,C) to a
            # [B,T,H,D] tensor: out row t' = h*128 + t//16 uses head h,
            # col c' = (t%16)*128 + d.  Y_slice = attn_h.reshape(128,
            # 16*128) @ out_w.T, contracting over (u=t%16, d).  One chunk
            # is one jb (512 out cols); chunks are emitted interleaved with
            # the NEXT head's attention so the PE-only projection hides the
            # exp-bound tail of the attention pipeline.
            def emit_op(at, bb, h, jb, last=False):
                av = at[:].rearrange("p (a u) -> p a u", u=16)
                psy = flowps.tile([128, 512], F32, tag="flow")
                for u in range(KC):
                    nc.tensor.matmul(
                        psy[:], av[:, :, u],
                        tow[:, u, jb * 512:(jb + 1) * 512],
                        start=(u == 0), stop=(u == KC - 1))
                # ys copy on DVE (projection must not touch the ACT queue,
                # which is saturated by exp during the paired attention);
                # y DMA rides the scalar queue so it never delays x
                # prefetch on the sync queue.
                for piece in range(2 if last else 1):
                    w0p = piece * 256 if last else 0
                    w1p = 256 + piece * 256 if last else 512
                    ys = ypool.tile([128, 512], F32, tag="ys")
                    nc.vector.tensor_copy(ys[:, w0p:w1p], psy[:, w0p:w1p])
                    # the final chunk's two pieces ride different DMA
                    # queues so their transfers overlap in the kernel tail
                    dq = nc.sync if (last and piece == 0) else nc.scalar
                    dq.dma_start(
                        y[bb * HPC + h, :, jb * 512 + w0p:jb * 512 + w1p],
                        ys[:, w0p:w1p])

            for b in range(B):
                # ---------------- QKV projection + RoPE ----------------
                # rots/vts are split per 512-token block so attention's
                # dependencies are block-granular (a monolithic tile would
                # make the first score matmul wait for the last RoPE chain).
                rots = [[rotpool.tile([128, TB], BF16, tag=f"rot{m}_{j}",
                                      name=f"rot{m}_{j}")
                         for j in range(NTB)] for m in range(4)]
                vts = [vpool.tile([128, 4, 2 * 128], BF16, tag=f"vt{j}",
                                  name=f"vt{j}") for j in range(NTB)]
                for tb in range(NTB):
                    c0 = b * T + tb * TB
                    ts_sl = slice(tb * TB, (tb + 1) * TB)
                    xTr = xT[:, c0:c0 + TB].rearrange(
                        "(kb p) t -> p kb t", p=128)
                    if b == 0 and tb == 0:
                        xgs = xg00  # preloaded across both DMA queues
                    else:
                        xgs = []
                        for g in range(KC // 2):
                            xg = xpool.tile([128, 2, TB], BF16, tag="xk",
                                            name="xg")
                            nc.sync.dma_start(xg[:],
                                              xTr[:, g * 2:(g + 1) * 2, :])
                            xgs.append(xg)
                    xk = [xgs[k // 2][:, k % 2, :] for k in range(KC)]
                    for m in range(4):
                        ps = flowps.tile([128, TB], F32, tag="flow")
                        for k in range(KC):
                            nc.tensor.matmul(
                                ps[:], twqk[:, k, m * 128:(m + 1) * 128],
                                xk[k], start=(k == 0), stop=(k == KC - 1))
                        # RoPE: rows 0:64 = x1, 64:128 = x2 of this head
                        qsb = tpool.tile([128, TB], BF16, tag="qsb")
                        nc.scalar.copy(qsb[:], ps[:])
                        if b == 0:
                            u = tb * 4 + m
                            nc.scalar.dma_start(tow[:, u, :], owr[:, u, :])
                        qsw = tpool.tile([128, TB], BF16, tag="qsw")
                        nc.gpsimd.dma_start(qsw[0:64, :], qsb[64:128, :])
                        nc.gpsimd.dma_start(qsw[64:128, :], qsb[0:64, :])
                        pc = tpool.tile([128, TB], BF16, tag="pc")
                        nc.vector.tensor_mul(out=pc[:], in0=qsb[:],
                                             in1=tcf[:, ts_sl])
                        pn = tpool.tile([128, TB], BF16, tag="pn")
                        nc.vector.tensor_mul(out=pn[:], in0=qsw[:],
                                             in1=tsn[:, ts_sl])
                        nc.vector.tensor_add(
                            out=rots[m][tb][:], in0=pc[:], in1=pn[:])
                    for vp in range(2):
                        psv = flowps.tile([128, TB], F32, tag="flow")
                        for half in range(2):
                            ts = vp * 2 + half
                            for k in range(KC):
                                nc.tensor.matmul(
                                    psv[:, half * 256:(half + 1) * 256],
                                    xk[k][:, ts * 128:(ts + 1) * 128],
                                    twv[:, k, :], start=(k == 0),
                                    stop=(k == KC - 1))
                        nc.vector.tensor_copy(
                            vts[tb][:, vp * 2:(vp + 1) * 2, :],
                            psv[:].rearrange("p (v c) -> p v c", v=2))

                # ---------------- attention ----------------
                atn = [apool.tile([128, T], BF16, tag=f"attnT{h}",
                                  name=f"attnT{h}") for h in range(HPC)]

                def emit_att(h, tb):
                    if True:
                        ts_sl = slice(tb * TB, (tb + 1) * TB)
                        ns = (tb + 1) * (TB // 128)
                        ng = ns // 4
                        ps_att = attps.tile([128, TB], F32, tag="psatt")
                        ps_den = denps.tile([128, TB], F32, tag="psden")
                        dacc = [None] * ng
                        den_ready = []

                        def den_mm(g):
                            nc.tensor.matmul(
                                ps_den[:], tones[:], dacc[g][:],
                                start=(g == 0), stop=(g == ng - 1))

                        def flush(ep, p, w):
                            # softmax denominator: group 4 et blocks with
                            # (partial-width) DVE adds, one ones-matmul per
                            # group, emitted one group late so the PE never
                            # waits on the DVE accumulation chain. Every
                            # group-start block is full-width (r <= 0).
                            g = p // 4
                            if p % 4 == 0:
                                dacc[g] = dpool.tile([128, TB], BF16,
                                                     tag="dacc", name="dacc")
                                nc.vector.tensor_copy(dacc[g][:], ep[:])
                            else:
                                nc.vector.tensor_add(
                                    out=dacc[g][:, w:], in0=dacc[g][:, w:],
                                    in1=ep[:, w:])
                            if p % 4 == 3:
                                den_ready.append(g)
                                if len(den_ready) > 1:
                                    den_mm(den_ready.pop(0))
                            nc.tensor.matmul(
                                ps_att[:, w:],
                                vts[p // 4][:, p % 4,
                                            h * 128:(h + 1) * 128],
                                ep[:, w:], start=(p == 0),
                                stop=(p == ns - 1))

                        pend = []
                        for si in range(ns):
                            # cols < r are fully causal-masked for this
                            # kt-block; compute only [w0:]
                            r = si * 128 - tb * TB
                            w0 = max(r, 0)
                            ps_sc = flowps.tile([128, TB], F32, tag="flow")
                            nc.tensor.matmul(
                                ps_sc[:, w0:],
                                rots[2 + h][si // 4][
                                    :, (si % 4) * 128:(si % 4 + 1) * 128],
                                rots[h][tb][:, w0:],
                                start=True, stop=True)
                            et = epool.tile([128, TB], BF16, tag="et")
                            nc.scalar.activation(
                                et[:, w0:], ps_sc[:, w0:],
                                mybir.ActivationFunctionType.Exp,
                                scale=SCALE)
                            if r >= 0:
                                # diagonal block: zero the sub-diagonal
                                # triangle of [w0:w0+128] by multiplying
                                # with a precomputed upper-triangle mask on
                                # DVE (keeps gpsimd off the attention
                                # critical path); cols [0:w0] stay garbage
                                # but every consumer reads [w0:] (the
                                # group's p%4==0 full-width dacc copy is
                                # always the r==0 block)
                                nc.vector.tensor_mul(
                                    out=et[:, w0:w0 + 128],
                                    in0=et[:, w0:w0 + 128],
                                    in1=tmask[:])
                            pend.append((et, si, w0))
                            if len(pend) > LAG:
                                flush(*pend.pop(0))
                        for args in pend:
                            flush(*args)
                        for g in den_ready:
                            den_mm(g)
                        rcp = rpool.tile([128, TB], F32, tag="rcp")
                        nc.vector.reciprocal_approx_fast(out=rcp[:],
                                                         in_=ps_den[:])
                        nc.vector.tensor_mul(
                            out=atn[h][:, ts_sl], in0=ps_att[:], in1=rcp[:])

                for h in range(HPC):
                    for tb in range(NTB):
                        emit_att(h, tb)
                for h in range(HPC):
                    for jb in range(4):
                        emit_op(atn[h], b, h, jb,
                                last=(b == B - 1 and h == HPC - 1
                                      and jb == 3))
    nc.compile()
    return nc


def _get_nc():
    global _CACHED_NC
    if _CACHED_NC is None:
        _CACHED_NC = build_nc()
    return _CACHED_NC


def _rope_tables():
    pos = np.arange(T, dtype=np.float64)[:, None]
    div = np.exp(np.arange(0, D, 2, dtype=np.float64) *
                 (-math.log(10000.0) / D))
    ang = pos * div  # [T, 64]
    sinT = np.sin(ang).T.astype(np.float32)  # [64, T]
    cosT = np.cos(ang).T.astype(np.float32)
    cosF = np.ascontiguousarray(np.concatenate([cosT, cosT], axis=0))
    sinS = np.ascontiguousarray(np.concatenate([-sinT, sinT], axis=0))
    return cosF.astype(NPBF), sinS.astype(NPBF)


def make_in_maps(x, qkv_w, out_w):
    xT = np.ascontiguousarray(x.reshape(BT, C).T).astype(NPBF)
    owF = np.ascontiguousarray(out_w.T).astype(NPBF)
    cosF, sinS = _rope_tables()
    ones = np.ones((128, 128), dtype=NPBF)
    mask = np.triu(np.ones((128, 128), dtype=np.float32)).astype(NPBF)
    in_maps = []
    for c in range(N_CORES):
        h0, h1 = 2 * c, 2 * c + 1
        wqk = np.concatenate([
            qkv_w[h0 * D:(h0 + 1) * D],
            qkv_w[h1 * D:(h1 + 1) * D],
            qkv_w[C + h0 * D:C + (h0 + 1) * D],
            qkv_w[C + h1 * D:C + (h1 + 1) * D],
        ], axis=0)                       # [512, 2048]
        wv = np.concatenate([
            qkv_w[2 * C + h0 * D:2 * C + (h0 + 1) * D],
            qkv_w[2 * C + h1 * D:2 * C + (h1 + 1) * D],
        ], axis=0)                       # [256, 2048]
        in_maps.append({
            "xT": xT,
            "wqkT": np.ascontiguousarray(wqk.T).astype(NPBF),
            "wvT": np.ascontiguousarray(wv.T).astype(NPBF),
            "owF": owF,
            "cosF": cosF,
            "sinS": sinS,
            "onesI": ones,
            "maskI": mask,
        })
    return in_maps


def kernel(x, qkv_w, out_w, _trace=False, _trace_kwargs=None):
    x = np.asarray(x, dtype=np.float32)
    qkv_w = np.asarray(qkv_w, dtype=np.float32)
    out_w = np.asarray(out_w, dtype=np.float32)
    nc = _get_nc()
    in_maps = make_in_maps(x, qkv_w, out_w)
    kwargs = {}
    if _trace:
        kwargs["trace"] = True
        if _trace_kwargs:
            kwargs.update(_trace_kwargs)
    res = run_bass_kernel_spmd(nc, in_maps, core_ids=list(range(N_CORES)),
                               **kwargs)
    out = np.empty((B, T, C), dtype=np.float32)
    for c in range(N_CORES):
        yc = res.results[c]["y"]  # [B*HPC, 128, C]
        for b in range(B):
            for hl in range(HPC):
                hg = HPC * c + hl
                out[b, hg * 128:(hg + 1) * 128] = yc[b * HPC + hl]
    if _trace:
        return out, res
    return out
